# revision 1
# baseline (speedup 1.0000x reference)
"""LoRA kernel for TRN2: y = (x @ A) @ B * scale, data-parallel over 8 cores.

Reference materializes W = (A@B)*scale [4096,4096] then x@W (~275 GFLOP).
Mathematically identical low-rank evaluation: u = x@(A*scale) [rows,8],
y = u@B — ~2 GFLOP, memory-bound.

Per-core plan (rows sharded 8192/8 = 1024 rows/core, A/B replicated):
  for each 128-row tile:
    DMA x tile [128, 4096] (2 MB contiguous)
    for each group of 4 feature-chunks:
      4x PE transpose x chunk [128r,128f] -> xT [128f,128r] into one PSUM bank
      1x DVE copy PSUM->SBUF [128, 512]
      4x PE matmul (lhsT = xT chunk stationary, rhs = A chunk [128,8] moving)
         accumulating u [128 rows, 8] in PSUM
    copy u -> SBUF, PE-transpose -> uT [8,128], copy -> SBUF
    8x PE matmul (lhsT = uT stationary, rhs = B chunk [8,512] moving)
       -> y chunks [128,512] in PSUM, ACT copies to SBUF
    DMA y tile [128, 4096] out (2 MB contiguous)
"""

import os

import numpy as np

os.environ.setdefault("MYCRO_LOCAL_CACHE", "1")

import concourse.bacc as bacc
import concourse.mybir as mybir
import concourse.tile as tile
from concourse.bass_utils import run_bass_kernel_spmd

F32 = mybir.dt.float32

N_CORES = 8
BATCH, SEQ, D = 4, 2048, 4096
RANK = 8
SCALE = 16 / 8
ROWS = BATCH * SEQ            # 8192
R_CORE = ROWS // N_CORES      # 1024 rows per core
P = 128                       # partitions
RT = R_CORE // P              # 8 row tiles per core
KC = D // P                   # 32 feature chunks
GRP = 4                       # chunks per PSUM staging bank
NB = D // 512                 # 8 output column chunks

# y matmul dtype: float32 (exact) or float32r (4x faster streaming, reduced
# precision multiplies). Toggled for experiments via build(y_dtype=...).
_NC_CACHE = {}


def build(y_dtype=F32, u_dtype=F32):
    nc = bacc.Bacc("TRN2", target_bir_lowering=False, debug=False)

    x_d = nc.dram_tensor("x", [R_CORE, D], F32, kind="ExternalInput")
    a_d = nc.dram_tensor("A", [D, RANK], F32, kind="ExternalInput")
    b_d = nc.dram_tensor("B", [RANK, D], F32, kind="ExternalInput")
    i_d = nc.dram_tensor("ident", [P, P], F32, kind="ExternalInput")
    y_d = nc.dram_tensor("y", [R_CORE, D], F32, kind="ExternalOutput")

    with tile.TileContext(nc) as tc:
        with (
            tc.tile_pool(name="const", bufs=1) as cpool,
            tc.tile_pool(name="xpool", bufs=3) as xpool,
            tc.tile_pool(name="xtpool", bufs=3) as xtpool,
            tc.tile_pool(name="ypool", bufs=2) as ypool,
            tc.tile_pool(name="small", bufs=2) as spool,
            tc.tile_pool(name="ps_xt", bufs=2, space="PSUM") as ps_xt,
            tc.tile_pool(name="ps_u", bufs=1, space="PSUM") as ps_u,
            tc.tile_pool(name="ps_ut", bufs=1, space="PSUM") as ps_ut,
            tc.tile_pool(name="ps_y", bufs=3, space="PSUM") as ps_y,
        ):
            ident = cpool.tile([P, P], F32)
            nc.sync.dma_start(ident[:], i_d[:, :])

            # A in [p, kc, r] layout: a_sb[p, kc, r] = A[kc*128 + p, r]
            a_sb = cpool.tile([P, KC, RANK], F32)
            nc.sync.dma_start(a_sb[:], a_d[:, :].rearrange("(kc p) r -> p kc r", p=P))

            b_sb = cpool.tile([RANK, D], F32)
            nc.sync.dma_start(b_sb[:], b_d[:, :])

            for t in range(RT):
                x_row = xpool.tile([P, D], F32, tag="x")
                nc.sync.dma_start(x_row[:], x_d[t * P:(t + 1) * P, :])

                u_ps = ps_u.tile([P, RANK], F32, tag="u")
                for g in range(KC // GRP):
                    xt_ps = ps_xt.tile([P, GRP * P], F32, tag="xt")
                    for j in range(GRP):
                        kc = g * GRP + j
                        nc.tensor.transpose(
                            xt_ps[:, j * P:(j + 1) * P],
                            x_row[:, kc * P:(kc + 1) * P],
                            ident[:],
                        )
                    xt_sb = xtpool.tile([P, GRP * P], F32, tag="xt_sb")
                    nc.vector.tensor_copy(xt_sb[:], xt_ps[:])
                    for j in range(GRP):
                        kc = g * GRP + j
                        lhsT = xt_sb[:, j * P:(j + 1) * P]
                        rhs = a_sb[:, kc, :]
                        if u_dtype != F32:
                            lhsT = lhsT.bitcast(u_dtype)
                            rhs = rhs.bitcast(u_dtype)
                        nc.tensor.matmul(
                            u_ps[:],
                            lhsT,
                            rhs,
                            start=(kc == 0),
                            stop=(kc == KC - 1),
                        )

                u_sb = spool.tile([P, RANK], F32, tag="u_sb")
                nc.vector.tensor_copy(u_sb[:], u_ps[:])
                ut_ps = ps_ut.tile([RANK, P], F32, tag="ut")
                nc.tensor.transpose(ut_ps[:], u_sb[:], ident[:])
                ut_sb = spool.tile([RANK, P], F32, tag="ut_sb")
                nc.vector.tensor_copy(ut_sb[:], ut_ps[:])

                y_sb = ypool.tile([P, D], F32, tag="y")
                for j in range(NB):
                    y_ps = ps_y.tile([P, 512], F32, tag="y_ps")
                    lhsT = ut_sb[:]
                    rhs = b_sb[:, j * 512:(j + 1) * 512]
                    if y_dtype != F32:
                        lhsT = lhsT.bitcast(y_dtype)
                        rhs = rhs.bitcast(y_dtype)
                    nc.tensor.matmul(y_ps[:], lhsT, rhs)
                    nc.scalar.copy(y_sb[:, j * 512:(j + 1) * 512], y_ps[:])

                nc.scalar.dma_start(y_d[t * P:(t + 1) * P, :], y_sb[:])

    nc.compile()
    return nc


def get_nc(y_dtype=F32, u_dtype=F32):
    key = (str(y_dtype), str(u_dtype))
    if key not in _NC_CACHE:
        _NC_CACHE[key] = build(y_dtype=y_dtype, u_dtype=u_dtype)
    return _NC_CACHE[key]


def _prep_in_maps(x, A, B):
    xf = np.ascontiguousarray(np.asarray(x), dtype=np.float32).reshape(ROWS, D)
    af = np.ascontiguousarray(np.asarray(A), dtype=np.float32) * np.float32(SCALE)
    bf = np.ascontiguousarray(np.asarray(B), dtype=np.float32)
    ident = np.eye(P, dtype=np.float32)
    return [
        {
            "x": xf[c * R_CORE:(c + 1) * R_CORE],
            "A": af,
            "B": bf,
            "ident": ident,
        }
        for c in range(N_CORES)
    ]


def kernel(x, A, B, _nc=None, **run_kwargs):
    nc = _nc if _nc is not None else get_nc()
    in_maps = _prep_in_maps(x, A, B)
    res = run_bass_kernel_spmd(nc, in_maps, core_ids=list(range(N_CORES)), **run_kwargs)
    y = np.concatenate([r["y"] for r in res.results], axis=0)
    out = y.reshape(BATCH, SEQ, D)
    if run_kwargs:
        return out, res
    return out
